# revision 4
# baseline (speedup 1.0000x reference)
"""GCN message-passing network on 8 Trainium2 NeuronCores (Bass/Tile).

Strategy:
  - ids is sorted -> graph g's nodes are contiguous; core c owns graph c
    (rows padded to NGP per core). Global max-pool becomes core-local.
  - Linearity: A@(xW) == (A@x)@W, so sparse layers aggregate raw h tables
    (bf16, 256B rows) and apply W post-aggregation.
  - Pooled layers (3 and 5) collapse to dense S @ (mx @ W): S[n,g] = sum of
    incoming edge weights from graph g (CPU-precomputed, exact).
  - Aggregation: per-node K=16 edge-slot grid (dma_gather bulk gathers with
    int16 indices -> the node table is split in 3 segments of <=32767 rows,
    grid columns statically partitioned per segment) + an overflow path for
    deg>K edges reduced on the PE via one-hot M matrices.
  - Tables h1/h3 are AllGathered compactly across cores, then restrided to
    256B rows for the next layer's gathers.

Falls back to a pure-numpy path if inputs don't match the expected
shape/distribution budgets.
"""

import numpy as np

# ---------------- problem constants ----------------
N, E, NG = 80000, 1280000, 8
F_IN, NC_CLS = 3, 20
BN_EPS = 1e-3
NCORES = 8

# ---------------- kernel configuration ----------------
CFG_FULL = dict(
    TPC=84,          # node tiles per core (128 nodes each)
    GB=6,            # tiles per gather batch
    MS=12,           # masked pool tiles at the tail of each core's range
    K_S=(6, 6, 4),   # grid columns per source segment
    B_OV=(2, 2, 2),  # overflow blocks (128 edges) per tile per segment
)

ELEM = 128           # bf16 elems per table row = 256 bytes


def _derive(cfg):
    TPC, GB = cfg["TPC"], cfg["GB"]
    d = dict(cfg)
    d["NGP"] = TPC * 128
    d["NP"] = d["NGP"] * NCORES
    d["NB"] = TPC // GB
    assert TPC % GB == 0
    # segments = core triples (0-2, 3-5, 6-7); each must be < 32768 rows
    d["SEG_STARTS"] = (0, 3 * d["NGP"], 6 * d["NGP"])
    d["SEG_ROWS"] = (3 * d["NGP"], 3 * d["NGP"], 2 * d["NGP"])
    assert max(d["SEG_ROWS"]) < 32768
    d["KOFF"] = (0, cfg["K_S"][0], cfg["K_S"][0] + cfg["K_S"][1])
    d["K"] = sum(cfg["K_S"])
    return d


# ---------------- numpy fallback ----------------

def _np_forward(x, edge_w, src, dst, ids,
                W1, b1, W2, b2, g1, be1, m1, v1,
                W3, b3, W4, b4, g2, be2, m2, v2,
                W5, b5):
    try:
        import scipy.sparse as sp
        A = sp.coo_matrix((edge_w, (dst, src)), shape=(x.shape[0], x.shape[0]),
                          dtype=np.float32).tocsr()
        spmm = lambda h: A @ h
    except ImportError:
        def spmm(h):
            out = np.zeros_like(h)
            np.add.at(out, dst, h[src] * edge_w[:, None])
            return out

    relu = lambda a: np.maximum(a, 0.0)
    bn = lambda h, g, be, m, v: (h - m) * (g / np.sqrt(v + BN_EPS)) + be

    def pool(h):
        mx = np.full((NG, h.shape[1]), -np.inf, dtype=np.float32)
        np.maximum.at(mx, ids, h)
        return mx[ids]

    h = relu(spmm(x) @ W1 + b1)
    h = relu(spmm(h) @ W2 + b2)
    h = pool(bn(h, g1, be1, m1, v1))
    h = relu(spmm(h) @ W3 + b3)
    h = relu(spmm(h) @ W4 + b4)
    h = pool(bn(h, g2, be2, m2, v2))
    z = spmm(h) @ W5 + b5
    z -= z.max(axis=-1, keepdims=True)
    ez = np.exp(z)
    return (ez / ez.sum(axis=-1, keepdims=True)).astype(np.float32)


# ---------------- CPU preprocessing ----------------

def _wrap_idx(flat):
    """[n] int16 position-ordered index list -> [128, n/16] wrapped array.

    Position i lives at (partition i%16, col i//16); rows 16-31 replicate
    rows 0-15 (the TX descriptor core reads its own partition window).
    """
    n = flat.shape[0]
    assert n % 16 == 0
    w = np.zeros((128, n // 16), dtype=np.int16)
    blk = flat.reshape(n // 16, 16).T        # [16, n/16]
    w[0:16] = blk
    w[16:32] = blk
    return w


def _preprocess(cfg, inputs):
    """Build per-core input arrays. Returns (in_maps, meta) or None if the
    data violates the static budgets (-> numpy fallback)."""
    import ml_dtypes
    bf16 = ml_dtypes.bfloat16

    TPC, GB, MS = cfg["TPC"], cfg["GB"], cfg["MS"]
    K_S, B_OV, KOFF, K = cfg["K_S"], cfg["B_OV"], cfg["KOFF"], cfg["K"]
    NGP, NP, NB = cfg["NGP"], cfg["NP"], cfg["NB"]
    SEG_STARTS = np.asarray(cfg["SEG_STARTS"], dtype=np.int64)

    ids = np.asarray(inputs["ids"]); src = np.asarray(inputs["src"])
    dst = np.asarray(inputs["dst"]); ew = np.asarray(inputs["edge_w"], dtype=np.float32)
    x = np.asarray(inputs["x"], dtype=np.float32)
    n_nodes = ids.shape[0]

    counts = np.bincount(ids, minlength=NG)
    if counts.max() > NGP or counts.min() < NGP - MS * 128 or not (np.diff(ids) >= 0).all():
        return None

    starts = np.concatenate([[0], np.cumsum(counts)])[:NG]
    offsets = np.arange(NG) * NGP - starts
    pad_map = np.arange(n_nodes, dtype=np.int64) + offsets[ids]

    src_p = pad_map[src]; dst_p = pad_map[dst]
    core_of = src_p // NGP  # src graph (used for S); dst core below
    dst_core = dst_p // NGP
    seg = (src_p >= SEG_STARTS[1]).astype(np.int64) + (src_p >= SEG_STARTS[2])
    rebased = src_p - SEG_STARTS[seg]
    assert rebased.max() < 32768

    in_maps = []
    KS_arr = np.asarray(K_S)

    # shared tables / constants
    x_tab = np.zeros((NP, ELEM), dtype=bf16)
    x_tab[pad_map, 0:x.shape[1]] = x.astype(bf16)
    iota_row = np.tile(np.arange(128, dtype=np.float32)[None, :], (128, 1))
    ident = np.eye(128, dtype=np.float32)

    def vec(v, rows):
        a = np.zeros((rows, 1), dtype=np.float32)
        a[: v.shape[0], 0] = v
        return a

    W1 = np.asarray(inputs["W1"], dtype=np.float32)
    w1p = np.zeros((4, 32), dtype=np.float32); w1p[0:3] = W1
    w2 = np.asarray(inputs["W2"], dtype=np.float32)
    w3 = np.asarray(inputs["W3"], dtype=np.float32)
    w4 = np.asarray(inputs["W4"], dtype=np.float32)
    w5 = np.asarray(inputs["W5"], dtype=np.float32)
    g1 = np.asarray(inputs["g1"], dtype=np.float32); v1 = np.asarray(inputs["v1"], dtype=np.float32)
    m1 = np.asarray(inputs["m1"], dtype=np.float32); be1 = np.asarray(inputs["be1"], dtype=np.float32)
    g2 = np.asarray(inputs["g2"], dtype=np.float32); v2 = np.asarray(inputs["v2"], dtype=np.float32)
    m2 = np.asarray(inputs["m2"], dtype=np.float32); be2 = np.asarray(inputs["be2"], dtype=np.float32)
    s1 = g1 / np.sqrt(v1 + BN_EPS); t1 = be1 - m1 * s1
    s2 = g2 / np.sqrt(v2 + BN_EPS); t2 = be2 - m2 * s2

    const_common = {
        "xtab": x_tab,
        "iota": iota_row, "ident": ident,
        "w1": w1p, "w2": w2, "w3": w3, "w4": w4, "w5": w5,
        "b1": vec(np.asarray(inputs["b1"], np.float32), 32),
        "b2": vec(np.asarray(inputs["b2"], np.float32), 32),
        "b3": vec(np.asarray(inputs["b3"], np.float32), 64),
        "b4": vec(np.asarray(inputs["b4"], np.float32), 64),
        "b5": vec(np.asarray(inputs["b5"], np.float32), 20),
        "s1": vec(s1, 32), "t1": vec(t1, 32),
        "s2": vec(s2, 64), "t2": vec(t2, 64),
    }

    for c in range(NCORES):
        sel = dst_core == c
        dl = (dst_p[sel] - c * NGP).astype(np.int64)
        sg = seg[sel]; rs = rebased[sel]; w_e = ew[sel]; sgr = core_of[sel]

        # rank of each edge within its (node, seg) group
        key = dl * 4 + sg
        order = np.argsort(key, kind="stable")
        ks = key[order]
        is_first = np.ones(len(ks), dtype=bool); is_first[1:] = ks[1:] != ks[:-1]
        first_pos = np.where(is_first, np.arange(len(ks)), 0)
        first_pos = np.maximum.accumulate(first_pos)
        rank = np.arange(len(ks)) - first_pos
        dl_o, sg_o, rs_o, w_o = dl[order], sg[order], rs[order], w_e[order]

        kcap = KS_arr[sg_o]
        ingrid = rank < kcap

        g_idx = np.zeros((NGP, K), dtype=np.int16)
        g_ew = np.zeros((NGP, K), dtype=np.float32)
        col = np.asarray(KOFF)[sg_o[ingrid]] + rank[ingrid]
        g_idx[dl_o[ingrid], col] = rs_o[ingrid]
        g_ew[dl_o[ingrid], col] = w_o[ingrid]

        # overflow edges -> per (tile, seg) lists
        ovm = ~ingrid
        ot = dl_o[ovm] // 128
        okey = ot * 4 + sg_o[ovm]
        oorder = np.argsort(okey, kind="stable")
        oks = okey[oorder]
        ofirst = np.ones(len(oks), dtype=bool)
        if len(oks) > 1:
            ofirst[1:] = oks[1:] != oks[:-1]
        ofp = np.where(ofirst, np.arange(len(oks)), 0)
        ofp = np.maximum.accumulate(ofp) if len(oks) else ofp
        orank = np.arange(len(oks)) - ofp
        o_t = ot[oorder]; o_s = sg_o[ovm][oorder]
        o_rs = rs_o[ovm][oorder]; o_w = w_o[ovm][oorder]
        o_ell = (dl_o[ovm][oorder] % 128).astype(np.float32)

        cm = {}
        for s in range(3):
            cap = B_OV[s] * 128
            msk = o_s == s
            if msk.any() and orank[msk].max() >= cap:
                return None
            ov_idx = np.zeros((TPC, B_OV[s] * 128), dtype=np.int16)
            ov_ew = np.zeros((TPC, B_OV[s] * 128), dtype=np.float32)
            ov_ell = np.zeros((TPC, B_OV[s] * 128), dtype=np.float32)
            ov_idx[o_t[msk], orank[msk]] = o_rs[msk]
            ov_ew[o_t[msk], orank[msk]] = o_w[msk]
            ov_ell[o_t[msk], orank[msk]] = o_ell[msk]

            # grid gather index list per batch: positions (tile_rel, k, p)
            gidx_t = g_idx[:, KOFF[s]:KOFF[s] + K_S[s]].reshape(TPC, 128, K_S[s])
            per_batch = gidx_t.reshape(NB, GB, 128, K_S[s]).transpose(0, 1, 3, 2)
            wrapped = np.concatenate(
                [_wrap_idx(per_batch[b].reshape(-1)) for b in range(NB)], axis=1)
            cm[f"gidx{s}"] = wrapped
            # ew grid [128, TPC*K_s]: col = t*K_s + k
            cm[f"gew{s}"] = np.ascontiguousarray(
                gidx_t_ew := g_ew[:, KOFF[s]:KOFF[s] + K_S[s]]
                .reshape(TPC, 128, K_S[s]).transpose(1, 0, 2).reshape(128, TPC * K_S[s])
            ).astype(bf16)

            # overflow gather list per batch: (tile_rel, block, p)
            oidx_t = ov_idx.reshape(NB, GB, B_OV[s], 128)
            cm[f"ovidx{s}"] = np.concatenate(
                [_wrap_idx(oidx_t[b].reshape(-1)) for b in range(NB)], axis=1)
            cm[f"ovew{s}"] = np.ascontiguousarray(
                ov_ew.reshape(TPC, B_OV[s], 128).transpose(2, 0, 1)
                .reshape(128, TPC * B_OV[s])).astype(bf16)
            cm[f"ovell{s}"] = np.ascontiguousarray(
                ov_ell.reshape(TPC, B_OV[s], 128).transpose(2, 0, 1)
                .reshape(128, TPC * B_OV[s]))

        # S^T [8, NGP]
        st = np.bincount(dl * NG + sgr, weights=w_e, minlength=NGP * NG)
        cm["stab"] = st.reshape(NGP, NG).T.astype(bf16)

        # pool mask for the last MS tiles: 0 for real nodes, -1e30 for pads
        nreal = counts[c]
        node_idx = np.arange((TPC - MS) * 128, TPC * 128)
        mrow = np.where(node_idx < nreal, 0.0, -1e30).astype(np.float32)
        cm["pmask"] = np.tile(mrow[None, :], (128, 1))

        cm.update(const_common)
        in_maps.append(cm)

    meta = dict(counts=counts, starts=starts)
    return in_maps, meta


# ---------------- Bass program ----------------

def _build_nc(cfg):
    import concourse.bass as bass
    import concourse.bacc as bacc
    import concourse.tile as tile
    import concourse.mybir as mybir
    from concourse.library_config import mlp

    TPC, GB, MS = cfg["TPC"], cfg["GB"], cfg["MS"]
    K_S, B_OV, KOFF, K = cfg["K_S"], cfg["B_OV"], cfg["KOFF"], cfg["K"]
    NGP, NP, NB = cfg["NGP"], cfg["NP"], cfg["NB"]
    SEG_STARTS, SEG_ROWS = cfg["SEG_STARTS"], cfg["SEG_ROWS"]
    BT = sum(B_OV)  # total overflow blocks per tile
    fp32, bf16, i16 = mybir.dt.float32, mybir.dt.bfloat16, mybir.dt.int16
    AT = mybir.ActivationFunctionType
    OP = mybir.AluOpType
    AX = mybir.AxisListType

    nc = bacc.Bacc("TRN2", target_bir_lowering=False, debug=False,
                   num_devices=NCORES)

    def din(name, shape, dt):
        return nc.dram_tensor(name, shape, dt, kind="ExternalInput").ap()

    xtab = din("xtab", [NP, ELEM], bf16)
    gidx = [din(f"gidx{s}", [128, NB * GB * K_S[s] * 8], i16) for s in range(3)]
    ovidx = [din(f"ovidx{s}", [128, NB * GB * B_OV[s] * 8], i16) for s in range(3)]
    gew = [din(f"gew{s}", [128, TPC * K_S[s]], bf16) for s in range(3)]
    ovew = [din(f"ovew{s}", [128, TPC * B_OV[s]], bf16) for s in range(3)]
    ovell = [din(f"ovell{s}", [128, TPC * B_OV[s]], fp32) for s in range(3)]
    stab_d = din("stab", [NG, NGP], bf16)
    pmask_d = din("pmask", [128, MS * 128], fp32)
    iota_d = din("iota", [128, 128], fp32)
    ident_d = din("ident", [128, 128], fp32)
    w_d = {k: din(k, shp, fp32) for k, shp in
           [("w1", [4, 32]), ("w2", [32, 32]), ("w3", [32, 64]),
            ("w4", [64, 64]), ("w5", [64, 20])]}
    v_d = {k: din(k, [r, 1], fp32) for k, r in
           [("b1", 32), ("b2", 32), ("b3", 64), ("b4", 64), ("b5", 20),
            ("s1", 32), ("t1", 32), ("s2", 64), ("t2", 64)]}
    out_d = nc.dram_tensor("out", [NGP, NC_CLS], fp32, kind="ExternalOutput").ap()

    with tile.TileContext(nc) as tc:
        with (
            tc.tile_pool(name="const", bufs=1) as cp,
            tc.tile_pool(name="work", bufs=2) as wp,
            tc.tile_pool(name="small", bufs=3) as sp,
            tc.tile_pool(name="psum", bufs=2, space="PSUM") as pp,
            tc.tile_pool(name="dram", bufs=1, space="DRAM") as dp,
        ):
            nc.gpsimd.load_library(mlp)

            # ---- resident constants ----
            def ld(ap_in, shape, dt, rows=None, tag=None):
                t = cp.tile(shape, dt, tag=tag)
                if rows is None:
                    nc.sync.dma_start(t[:], ap_in)
                else:
                    nc.sync.dma_start(t[0:rows, :], ap_in)
                return t

            gidx_sb = [ld(gidx[s], [128, NB * GB * K_S[s] * 8], i16, tag=f"c_gidx{s}") for s in range(3)]
            ovidx_sb = [ld(ovidx[s], [128, NB * GB * B_OV[s] * 8], i16, tag=f"c_ovidx{s}") for s in range(3)]
            gew_sb = [ld(gew[s], [128, TPC * K_S[s]], bf16, tag=f"c_gew{s}") for s in range(3)]
            ovew_sb = [ld(ovew[s], [128, TPC * B_OV[s]], bf16, tag=f"c_ovew{s}") for s in range(3)]
            ovell_sb = [ld(ovell[s], [128, TPC * B_OV[s]], fp32, tag=f"c_ovell{s}") for s in range(3)]
            stab_sb = ld(stab_d, [128, NGP], bf16, rows=NG, tag="c_stab")
            pmask_sb = ld(pmask_d, [128, MS * 128], fp32, tag="c_pmask")
            iota_sb = ld(iota_d, [128, 128], fp32, tag="c_iota")
            ident_sb = ld(ident_d, [128, 128], fp32, tag="c_ident")
            w_sb = {}
            for k, shp in [("w1", [4, 32]), ("w2", [32, 32]), ("w3", [32, 64]),
                           ("w4", [64, 64]), ("w5", [64, 20])]:
                t = cp.tile([128, shp[1]], fp32, tag=f"c_w_{k}")
                nc.sync.dma_start(t[0:shp[0], :], w_d[k])
                w_sb[k] = t
            v_sb = {}
            for k, r in [("b1", 32), ("b2", 32), ("b3", 64), ("b4", 64),
                         ("b5", 20), ("s1", 32), ("t1", 32), ("s2", 64), ("t2", 64)]:
                t = cp.tile([128, 1], fp32, tag=f"c_v_{k}")
                nc.sync.dma_start(t[0:r, :], v_d[k])
                v_sb[k] = t

            # pool accumulators
            acc1 = cp.tile([128, 1], fp32, tag="c_acc1"); nc.vector.memset(acc1[:], -1e30)
            acc2 = cp.tile([128, 1], fp32, tag="c_acc2"); nc.vector.memset(acc2[:], -1e30)

            # dram intermediates
            t2_shard = dp.tile([NGP, 32], bf16)
            t2_full = dp.tile([NP, 32], bf16)
            t2_pad = dp.tile([NP, ELEM], bf16)
            t4_shard = dp.tile([NGP, 64], bf16)
            t4_full = dp.tile([NP, 64], bf16)
            t4_pad = dp.tile([NP, ELEM], bf16)
            mx1_sh = dp.tile([1, 32], fp32); mx1_all = dp.tile([NG, 32], fp32)
            mx2_sh = dp.tile([1, 64], fp32); mx2_all = dp.tile([NG, 64], fp32)

            RG = list(range(NCORES))

            def seg_view(table_ap):
                return [table_ap[SEG_STARTS[s]:SEG_STARTS[s] + SEG_ROWS[s], :]
                        for s in range(3)]

            def sparse_layer(table_ap, fin, fout, wkey, wrows, post):
                """Aggregate A@table (fin cols), multiply W [wrows,fout], call
                post(t, zT_psum) with zT [fout,128] psum."""
                segs = seg_view(table_ap)
                for b in range(NB):
                    greg, ovreg = [], []
                    for s in range(3):
                        n_g = GB * K_S[s] * 128
                        g = wp.tile([128, GB * K_S[s] * ELEM], bf16, tag=f"g{s}")
                        nc.gpsimd.dma_gather(
                            g[:].rearrange("p (k e) -> p k e", e=ELEM),
                            segs[s], gidx_sb[s][:, b * (n_g // 16):(b + 1) * (n_g // 16)],
                            n_g, n_g, ELEM, single_packet=False)
                        greg.append(g)
                        n_o = GB * B_OV[s] * 128
                        o = wp.tile([128, GB * B_OV[s] * ELEM], bf16, tag=f"o{s}")
                        nc.gpsimd.dma_gather(
                            o[:].rearrange("p (k e) -> p k e", e=ELEM),
                            segs[s], ovidx_sb[s][:, b * (n_o // 16):(b + 1) * (n_o // 16)],
                            n_o, n_o, ELEM, single_packet=False)
                        ovreg.append(o)

                    for tt in range(GB):
                        t = b * GB + tt
                        # ---- overflow path: masks + matmul accumulate ----
                        pov = pp.tile([128, 64], fp32, tag="psA")
                        mall = sp.tile([128, BT * 128], bf16, tag="mall")
                        gwo = sp.tile([128, BT * 64], bf16, tag="gwo")
                        blk = 0
                        for s in range(3):
                            bs = B_OV[s]
                            # masks for this tile's blocks of segment s
                            nc.vector.tensor_tensor(
                                out=mall[:, blk * 128:(blk + bs) * 128],
                                in0=bass.AP(iota_sb.tensor, iota_sb[:].offset,
                                            [[iota_sb[:].ap[0][0], 128], [0, bs], [1, 128]]),
                                in1=bass.AP(ovell_sb[s].tensor,
                                            ovell_sb[s][:, t * bs:(t + 1) * bs].offset,
                                            [[ovell_sb[s][:].ap[0][0], 128], [1, bs], [0, 128]]),
                                op=OP.is_equal)
                            # weighted gathered rows (first fin cols)
                            ov3 = ovreg[s][:].rearrange("p (k e) -> p k e", e=ELEM)
                            nc.vector.tensor_tensor(
                                out=gwo[:].rearrange("p (k e) -> p k e", e=64)[:, blk:blk + bs, 0:fin],
                                in0=ov3[:, tt * bs:(tt + 1) * bs, 0:fin],
                                in1=ovew_sb[s][:, t * bs:(t + 1) * bs].to_broadcast(
                                    [128, bs, fin]),
                                op=OP.mult)
                            blk += bs
                        g3w = gwo[:].rearrange("p (k e) -> p k e", e=64)
                        m3 = mall[:].rearrange("p (k e) -> p k e", e=128)
                        for j in range(BT):
                            nc.tensor.matmul(
                                out=pov[:, 0:fin],
                                lhsT=m3[:, j, :],
                                rhs=g3w[:, j, 0:fin],
                                start=(j == 0), stop=(j == BT - 1))

                        # ---- grid path: weighted sums per segment ----
                        parts = []
                        for s in range(3):
                            ksz = K_S[s]
                            g3 = greg[s][:].rearrange("p (k e) -> p k e", e=ELEM)
                            gw = sp.tile([128, ksz * 64], bf16, tag=f"gw{s}")
                            gw3 = gw[:].rearrange("p (k e) -> p k e", e=64)
                            nc.vector.tensor_tensor(
                                out=gw3[:, 0:ksz, 0:fin],
                                in0=g3[:, tt * ksz:(tt + 1) * ksz, 0:fin],
                                in1=gew_sb[s][:, t * ksz:(t + 1) * ksz].to_broadcast(
                                    [128, ksz, fin]),
                                op=OP.mult)
                            # tree-reduce the k columns (views in 64-col units)
                            if ksz == 6:
                                t3 = sp.tile([128, 3 * 64], bf16, tag=f"t3{s}")
                                nc.vector.tensor_tensor(
                                    out=t3[:].rearrange("p (k e) -> p k e", e=64)[:, 0:3, 0:fin],
                                    in0=gw3[:, 0:3, 0:fin], in1=gw3[:, 3:6, 0:fin], op=OP.add)
                                t33 = t3[:].rearrange("p (k e) -> p k e", e=64)
                                t1b = sp.tile([128, 64], bf16, tag=f"t1{s}")
                                nc.vector.tensor_tensor(
                                    out=t1b[:].rearrange("p (k e) -> p k e", e=64)[:, 0:1, 0:fin],
                                    in0=t33[:, 0:1, 0:fin], in1=t33[:, 1:2, 0:fin], op=OP.add)
                                ssum = sp.tile([128, 64], bf16, tag=f"ss{s}")
                                nc.vector.tensor_tensor(
                                    out=ssum[:].rearrange("p (k e) -> p k e", e=64)[:, 0:1, 0:fin],
                                    in0=t1b[:].rearrange("p (k e) -> p k e", e=64)[:, 0:1, 0:fin],
                                    in1=t33[:, 2:3, 0:fin], op=OP.add)
                                parts.append(ssum)
                            else:  # ksz == 4
                                t2b = sp.tile([128, 2 * 64], bf16, tag=f"t2{s}")
                                nc.vector.tensor_tensor(
                                    out=t2b[:].rearrange("p (k e) -> p k e", e=64)[:, 0:2, 0:fin],
                                    in0=gw3[:, 0:2, 0:fin], in1=gw3[:, 2:4, 0:fin], op=OP.add)
                                t23 = t2b[:].rearrange("p (k e) -> p k e", e=64)
                                ssum = sp.tile([128, 64], bf16, tag=f"ss{s}")
                                nc.vector.tensor_tensor(
                                    out=ssum[:].rearrange("p (k e) -> p k e", e=64)[:, 0:1, 0:fin],
                                    in0=t23[:, 0:1, 0:fin], in1=t23[:, 1:2, 0:fin], op=OP.add)
                                parts.append(ssum)
                        c01 = sp.tile([128, 64], fp32, tag="c01")
                        nc.vector.tensor_tensor(
                            out=c01[:, 0:fin], in0=parts[0][:, 0:fin],
                            in1=parts[1][:, 0:fin], op=OP.add)
                        c012 = sp.tile([128, 64], fp32, tag="c012")
                        nc.vector.tensor_tensor(
                            out=c012[:, 0:fin], in0=c01[:, 0:fin],
                            in1=parts[2][:, 0:fin], op=OP.add)
                        agg = sp.tile([128, 64], fp32, tag="agg")
                        nc.vector.tensor_tensor(
                            out=agg[:, 0:fin], in0=c012[:, 0:fin],
                            in1=pov[:, 0:fin], op=OP.add)

                        # ---- transpose + W ----
                        aggT_ps = pp.tile([128, 128], fp32, tag="psB")
                        nc.tensor.transpose(out=aggT_ps[0:fin, :], in_=agg[:, 0:fin],
                                            identity=ident_sb[:])
                        aggT = sp.tile([128, 128], fp32, tag="aggTs")
                        nc.vector.tensor_copy(out=aggT[0:fin, :], in_=aggT_ps[0:fin, :])
                        zT = pp.tile([128, 128], fp32, tag="psC")
                        nc.tensor.matmul(out=zT[0:fout, :],
                                         lhsT=w_sb[wkey][0:wrows, 0:fout],
                                         rhs=aggT[0:fin, :], start=True, stop=True)
                        post(t, zT)

            # ---------- post hooks ----------
            def table_write(t, hT_sb, fout, shard):
                h_ps = pp.tile([128, 128], fp32, tag="psD")
                nc.tensor.transpose(out=h_ps[0:128, 0:fout], in_=hT_sb[0:fout, :],
                                    identity=ident_sb[0:fout, 0:fout])
                h_bf = sp.tile([128, 64], bf16, tag="hbf")
                nc.vector.tensor_copy(out=h_bf[:, 0:fout], in_=h_ps[:, 0:fout])
                nc.sync.dma_start(shard[t * 128:(t + 1) * 128, :], h_bf[:, 0:fout])

            def post_l1(t, zT):
                hT = sp.tile([128, 128], fp32, tag="hT")
                nc.scalar.activation(out=hT[0:32, :], in_=zT[0:32, :],
                                     func=AT.Relu, bias=v_sb["b1"][0:32, :])
                table_write(t, hT, 32, t2_shard)

            def post_l2(t, zT):
                hT = sp.tile([128, 128], fp32, tag="hT")
                nc.scalar.activation(out=hT[0:32, :], in_=zT[0:32, :],
                                     func=AT.Relu, bias=v_sb["b2"][0:32, :])
                qT = sp.tile([128, 128], fp32, tag="qT")
                nc.scalar.activation(out=qT[0:32, :], in_=hT[0:32, :],
                                     func=AT.Identity, bias=v_sb["t1"][0:32, :],
                                     scale=v_sb["s1"][0:32, :])
                if t >= TPC - MS:
                    mc = (t - (TPC - MS)) * 128
                    nc.vector.tensor_tensor(out=qT[0:32, :], in0=qT[0:32, :],
                                            in1=pmask_sb[0:32, mc:mc + 128], op=OP.add)
                tmax = sp.tile([128, 1], fp32, tag="tmax")
                nc.vector.tensor_reduce(out=tmax[0:32, :], in_=qT[0:32, :],
                                        axis=AX.X, op=OP.max)
                nc.vector.tensor_tensor(out=acc1[0:32, :], in0=acc1[0:32, :],
                                        in1=tmax[0:32, :], op=OP.max)

            def post_l4(t, zT):
                hT = sp.tile([128, 128], fp32, tag="hT")
                nc.scalar.activation(out=hT[0:64, :], in_=zT[0:64, :],
                                     func=AT.Relu, bias=v_sb["b4"][0:64, :])
                qT = sp.tile([128, 128], fp32, tag="qT")
                nc.scalar.activation(out=qT[0:64, :], in_=hT[0:64, :],
                                     func=AT.Identity, bias=v_sb["t2"][0:64, :],
                                     scale=v_sb["s2"][0:64, :])
                if t >= TPC - MS:
                    mc = (t - (TPC - MS)) * 128
                    nc.vector.tensor_tensor(out=qT[0:64, :], in0=qT[0:64, :],
                                            in1=pmask_sb[0:64, mc:mc + 128], op=OP.add)
                tmax = sp.tile([128, 1], fp32, tag="tmax")
                nc.vector.tensor_reduce(out=tmax[0:64, :], in_=qT[0:64, :],
                                        axis=AX.X, op=OP.max)
                nc.vector.tensor_tensor(out=acc2[0:64, :], in0=acc2[0:64, :],
                                        in1=tmax[0:64, :], op=OP.max)

            # ---------- layer 1 ----------
            sparse_layer(xtab, 4, 32, "w1", 4, post_l1)
            nc.gpsimd.collective_compute(
                "AllGather", mybir.AluOpType.bypass, replica_groups=[RG],
                ins=[t2_shard.opt()], outs=[t2_full.opt()])
            nc.sync.dma_start(t2_pad[:, 0:32], t2_full[:])

            # ---------- layer 2 + pool1 ----------
            sparse_layer(t2_pad[:], 32, 32, "w2", 32, post_l2)
            nc.sync.dma_start(mx1_sh[:], acc1[0:32, :])
            nc.gpsimd.collective_compute(
                "AllGather", mybir.AluOpType.bypass, replica_groups=[RG],
                ins=[mx1_sh.opt()], outs=[mx1_all.opt()])
            mx1 = sp.tile([128, 32], fp32, tag="mx")
            nc.sync.dma_start(mx1[0:NG, :], mx1_all[:])
            mxT_ps = pp.tile([128, 128], fp32, tag="psB")
            nc.tensor.transpose(out=mxT_ps[0:32, 0:NG], in_=mx1[0:NG, 0:32],
                                identity=ident_sb[0:NG, 0:NG])
            mxT = sp.tile([128, 8], fp32, tag="mxTs")
            nc.vector.tensor_copy(out=mxT[0:32, :], in_=mxT_ps[0:32, 0:NG])
            y3T_ps = pp.tile([128, 8], fp32, tag="psA")
            nc.tensor.matmul(out=y3T_ps[0:64, :], lhsT=w_sb["w3"][0:32, 0:64],
                             rhs=mxT[0:32, 0:NG], start=True, stop=True)
            y3T = sp.tile([128, 8], fp32, tag="y3Ts")
            nc.vector.tensor_copy(out=y3T[0:64, :], in_=y3T_ps[0:64, :])
            y3_ps = pp.tile([128, 64], fp32, tag="psD")
            nc.tensor.transpose(out=y3_ps[0:NG, 0:64], in_=y3T[0:64, 0:NG],
                                identity=ident_sb[0:64, 0:64])
            y3 = sp.tile([128, 64], bf16, tag="y3s")
            nc.vector.tensor_copy(out=y3[0:NG, :], in_=y3_ps[0:NG, 0:64])

            # ---------- layer 3 (dense S path) ----------
            for t in range(TPC):
                h3T_ps = pp.tile([128, 128], fp32, tag="psC")
                nc.tensor.matmul(out=h3T_ps[0:64, :], lhsT=y3[0:NG, 0:64],
                                 rhs=stab_sb[0:NG, t * 128:(t + 1) * 128],
                                 start=True, stop=True)
                h3T = sp.tile([128, 128], fp32, tag="hT")
                nc.scalar.activation(out=h3T[0:64, :], in_=h3T_ps[0:64, :],
                                     func=AT.Relu, bias=v_sb["b3"][0:64, :])
                table_write(t, h3T, 64, t4_shard)

            nc.gpsimd.collective_compute(
                "AllGather", mybir.AluOpType.bypass, replica_groups=[RG],
                ins=[t4_shard.opt()], outs=[t4_full.opt()])
            nc.sync.dma_start(t4_pad[:, 0:64], t4_full[:])

            # ---------- layer 4 + pool2 ----------
            sparse_layer(t4_pad[:], 64, 64, "w4", 64, post_l4)
            nc.sync.dma_start(mx2_sh[:], acc2[0:64, :])
            nc.gpsimd.collective_compute(
                "AllGather", mybir.AluOpType.bypass, replica_groups=[RG],
                ins=[mx2_sh.opt()], outs=[mx2_all.opt()])
            mx2 = sp.tile([128, 64], fp32, tag="mx")
            nc.sync.dma_start(mx2[0:NG, :], mx2_all[:])
            mx2T_ps = pp.tile([128, 128], fp32, tag="psB")
            nc.tensor.transpose(out=mx2T_ps[0:64, 0:NG], in_=mx2[0:NG, 0:64],
                                identity=ident_sb[0:NG, 0:NG])
            mx2T = sp.tile([128, 8], fp32, tag="mxTs")
            nc.vector.tensor_copy(out=mx2T[0:64, :], in_=mx2T_ps[0:64, 0:NG])
            y5T_ps = pp.tile([128, 8], fp32, tag="psA")
            nc.tensor.matmul(out=y5T_ps[0:20, :], lhsT=w_sb["w5"][0:64, 0:20],
                             rhs=mx2T[0:64, 0:NG], start=True, stop=True)
            y5T = sp.tile([128, 8], fp32, tag="y3Ts")
            nc.vector.tensor_copy(out=y5T[0:20, :], in_=y5T_ps[0:20, :])
            y5_ps = pp.tile([128, 64], fp32, tag="psD")
            nc.tensor.transpose(out=y5_ps[0:NG, 0:20], in_=y5T[0:20, 0:NG],
                                identity=ident_sb[0:20, 0:20])
            y5 = sp.tile([128, 64], bf16, tag="y3s")
            nc.vector.tensor_copy(out=y5[0:NG, 0:20], in_=y5_ps[0:NG, 0:20])

            # ---------- layer 5 + softmax ----------
            for t in range(TPC):
                lT_ps = pp.tile([128, 128], fp32, tag="psC")
                nc.tensor.matmul(out=lT_ps[0:20, :], lhsT=y5[0:NG, 0:20],
                                 rhs=stab_sb[0:NG, t * 128:(t + 1) * 128],
                                 start=True, stop=True)
                lT = sp.tile([128, 128], fp32, tag="hT")
                nc.scalar.activation(out=lT[0:20, :], in_=lT_ps[0:20, :],
                                     func=AT.Identity, bias=v_sb["b5"][0:20, :])
                l_ps = pp.tile([128, 32], fp32, tag="psD")
                nc.tensor.transpose(out=l_ps[0:128, 0:20], in_=lT[0:20, :],
                                    identity=ident_sb[0:20, 0:20])
                lg = sp.tile([128, 32], fp32, tag="lgs")
                nc.vector.tensor_copy(out=lg[:, 0:20], in_=l_ps[:, 0:20])
                nm = sp.tile([128, 1], fp32, tag="nm")
                nc.vector.tensor_reduce(out=nm[:], in_=lg[:, 0:20], axis=AX.X,
                                        op=OP.max, negate=True)
                ex = sp.tile([128, 32], fp32, tag="ex")
                nc.scalar.activation(out=ex[:, 0:20], in_=lg[:, 0:20],
                                     func=AT.Exp, bias=nm[:])
                sm = sp.tile([128, 1], fp32, tag="sm")
                nc.vector.tensor_reduce(out=sm[:], in_=ex[:, 0:20], axis=AX.X,
                                        op=OP.add)
                rc = sp.tile([128, 1], fp32, tag="rc")
                nc.vector.reciprocal(out=rc[:], in_=sm[:])
                ot = sp.tile([128, 32], fp32, tag="ot")
                nc.vector.tensor_scalar(out=ot[:, 0:20], in0=ex[:, 0:20],
                                        scalar1=rc[:], scalar2=None, op0=OP.mult)
                nc.sync.dma_start(out_d[t * 128:(t + 1) * 128, :], ot[:, 0:20])

    nc.finalize()
    return nc


_CACHE = {}


def _bass_forward(cfg_key, cfg, inputs):
    from concourse.bass_utils import run_bass_kernel_spmd

    pre = _preprocess(cfg, inputs)
    if pre is None:
        return None
    in_maps, meta = pre

    if cfg_key not in _CACHE:
        _CACHE[cfg_key] = _build_nc(cfg)
    nc = _CACHE[cfg_key]

    res = run_bass_kernel_spmd(nc, in_maps, core_ids=list(range(NCORES)))
    counts, starts = meta["counts"], meta["starts"]
    n_nodes = int(counts.sum())
    out = np.empty((n_nodes, NC_CLS), dtype=np.float32)
    for c in range(NCORES):
        out[starts[c]:starts[c] + counts[c]] = res.results[c]["out"][:counts[c]]
    return out


def kernel(x, edge_w, src, dst, ids,
           W1, b1, W2, b2, g1, be1, m1, v1,
           W3, b3, W4, b4, g2, be2, m2, v2,
           W5, b5):
    args = dict(x=np.asarray(x, np.float32), edge_w=np.asarray(edge_w, np.float32),
                src=np.asarray(src), dst=np.asarray(dst), ids=np.asarray(ids),
                W1=W1, b1=b1, W2=W2, b2=b2, g1=g1, be1=be1, m1=m1, v1=v1,
                W3=W3, b3=b3, W4=W4, b4=b4, g2=g2, be2=be2, m2=m2, v2=v2,
                W5=W5, b5=b5)
    if args["x"].shape == (N, F_IN) and args["src"].shape == (E,):
        try:
            out = _bass_forward("full", _derive(CFG_FULL), args)
            if out is not None:
                return out
        except Exception:
            import traceback
            traceback.print_exc()
    f32 = {k: np.asarray(v, np.float32) for k, v in args.items()
           if k not in ("src", "dst", "ids")}
    return _np_forward(src=args["src"], dst=args["dst"], ids=args["ids"], **f32)
